# revision 8
# baseline (speedup 1.0000x reference)
"""Criss-cross attention TRN2 kernel.

B=8 images, one per NeuronCore (data-parallel over batch).
H=W=160, C=64, CQ=8. Joint softmax over row+column scores computed as two
identical "row attention" passes — the column pass runs on the spatially
transposed image (with the diagonal mask) and stages its numerators/sums
to DRAM; the row pass adds them, normalizes, adds x. Softmax without
max-subtraction (|E| < ~40 is safe in fp32/bf16). gamma folded into Wv.

All SBUF working tiles are streamed per quad (4 rows), so SBUF stays tiny.
Scores/PV matmuls in bf16; accumulation fp32 in PSUM.
"""

import sys

for p in ("/opt/trn_rl_repo", "/root/.axon_site/_ro/trn_rl_repo"):
    if p not in sys.path:
        sys.path.insert(0, p)

import numpy as np

import concourse.bacc as bacc
import concourse.bass as bass
import concourse.tile as tile
from concourse import mybir
from concourse import bass_utils

B, H, W, C = 8, 160, 160, 64
CQ = 8
NPOS = H * W
CV = C + 1  # v channels + ones column (sum-of-P trick)
NQ = H // 4  # 40 quads of 4 rows

FP32 = mybir.dt.float32
BF16 = mybir.dt.bfloat16
AF = mybir.ActivationFunctionType

# quad-relative output pieces: row q of a quad covers quad-positions
# q*160..q*160+159 -> (w0, w1, chunk, part0) with part0 in {0,32,64,96}
PIECES = {
    0: [(0, 128, 0, 0), (128, 160, 1, 0)],
    1: [(0, 32, 1, 32), (32, 96, 1, 64), (96, 160, 2, 0)],
    2: [(0, 64, 2, 64), (64, 160, 3, 0)],
    3: [(0, 32, 3, 96), (32, 160, 4, 0)],
}

_cache = {}


def _build():
    if "nc" in _cache:
        return _cache["nc"]

    nc = bacc.Bacc("TRN2", target_bir_lowering=False, debug=False)

    # pass-2 (row) inputs: (h w)-major; pass-1 (col) inputs: (w h)-major
    x_d = nc.dram_tensor("x_in", [NPOS, C], FP32, kind="ExternalInput").ap()
    xT_d = nc.dram_tensor("xT_in", [C, NPOS], BF16, kind="ExternalInput").ap()
    xTt_d = nc.dram_tensor("xTt_in", [C, NPOS], BF16, kind="ExternalInput").ap()
    qT_d = nc.dram_tensor("qT_in", [CQ, NPOS], BF16, kind="ExternalInput").ap()
    kT_d = nc.dram_tensor("kT_in", [CQ, NPOS], BF16, kind="ExternalInput").ap()
    qTt_d = nc.dram_tensor("qTt_in", [CQ, NPOS], BF16, kind="ExternalInput").ap()
    kTt_d = nc.dram_tensor("kTt_in", [CQ, NPOS], BF16, kind="ExternalInput").ap()
    wv_d = nc.dram_tensor("wv", [C, C], BF16, kind="ExternalInput").ap()
    mkm_d = nc.dram_tensor("mask_main", [128, H], BF16, kind="ExternalInput").ap()
    mkt_d = nc.dram_tensor("mask_tail", [128, H], BF16, kind="ExternalInput").ap()
    out_d = nc.dram_tensor("out", [NPOS, C], FP32, kind="ExternalOutput").ap()
    # column-pass staging, (h w)-major so the row pass reads it contiguously
    oc_d = nc.dram_tensor("oc_stage", [H, W, CV], FP32, kind="Internal").ap()
    oc_flat = oc_d.rearrange("h w c -> (h w c)")

    with tile.TileContext(nc) as tc:
        with (
            tc.tile_pool(name="cst", bufs=1) as cst,
            tc.tile_pool(name="st", bufs=3) as st,
            tc.tile_pool(name="io", bufs=2) as io,
            tc.tile_pool(name="ps", bufs=1, space="PSUM") as ps,
        ):
            wv = cst.tile([C, C], BF16)
            nc.sync.dma_start(wv[:], wv_d[:])
            mkm = cst.tile([128, H], BF16)
            nc.sync.dma_start(mkm[:], mkm_d[:])
            mkt = cst.tile([128, H], BF16)
            nc.sync.dma_start(mkt[:], mkt_d[:])

            def attention_pass(xT_src, qT_src, kT_src, masked, finish):
                """One direction of criss-cross attention.

                Per quad a (4 "rows" of 160 positions): build V tiles for
                these rows, scores E^T = K^T-chunks x Q, exp (bf16), PV with
                ones-column giving numerators + sums in PSUM [128,5,CV];
                finish(a, por) consumes the PSUM quad.
                """
                for a in range(NQ):
                    base = 4 * a * 160
                    # stream this quad's inputs
                    xq = st.tile([C, 4, 160], BF16, name="xq", tag="xq")
                    nc.sync.dma_start(
                        xq[:],
                        xT_src[:, base : base + 640].rearrange(
                            "c (q p) -> c q p", p=160
                        ),
                    )
                    qq = st.tile([CQ, 4, 160], BF16, name="qq", tag="qq")
                    nc.sync.dma_start(
                        qq[:],
                        qT_src[:, base : base + 640].rearrange(
                            "c (q p) -> c q p", p=160
                        ),
                    )
                    kq = st.tile([CQ, 4, 160], BF16, name="kq", tag="kq")
                    nc.sync.dma_start(
                        kq[:],
                        kT_src[:, base : base + 640].rearrange(
                            "c (q p) -> c q p", p=160
                        ),
                    )

                    # V for the 4 rows: main [128,4,CV] + packed tails [128,CV]
                    pv_b = ps.tile([128, 512], FP32, name="pv_b", tag="pv", bufs=2)
                    pv = pv_b[:, 0 : 4 * C + C]
                    for q in range(4):
                        nc.tensor.matmul(
                            pv[:, q * C : (q + 1) * C], xq[:, q, 0:128], wv[:],
                            start=True, stop=True,
                        )
                        nc.tensor.matmul(
                            pv[32 * q : 32 * (q + 1), 4 * C : 5 * C],
                            xq[:, q, 128:160], wv[:],
                            start=True, stop=True, tile_position=(0, 32 * q),
                        )
                    vm = st.tile([128, 4, CV], BF16, name="vm", tag="vm")
                    vt = st.tile([128, CV], BF16, name="vt", tag="vt")
                    pv5 = pv[:, 0 : 4 * C].rearrange("p (q c) -> p q c", c=C)
                    nc.vector.tensor_copy(vm[:, :, 0:C], pv5[:])
                    nc.vector.tensor_copy(vt[:, 0:C], pv[:, 4 * C : 5 * C])
                    nc.vector.memset(vm[:, :, C], 1.0)
                    nc.vector.memset(vt[:, C : C + 1], 1.0)

                    # scores: E^T [k, pos] per row; rows packed 2-per-bank
                    pe01_b = ps.tile([128, 512], FP32, name="pe01_b", tag="pe01", bufs=1)
                    pe01 = pe01_b[:, 0 : 2 * H].rearrange("p (q h) -> p q h", h=H)
                    pe23_b = ps.tile([128, 512], FP32, name="pe23_b", tag="pe23", bufs=1)
                    pe23 = pe23_b[:, 0 : 2 * H].rearrange("p (q h) -> p q h", h=H)
                    pet_b = ps.tile([128, 512], FP32, name="pet_b", tag="pet", bufs=1)
                    pet = pet_b[:, 0:H]
                    for q in range(4):
                        peq = (pe01, pe23)[q // 2][:, q % 2, :]
                        nc.tensor.matmul(
                            peq, kq[:, q, 0:128], qq[:, q, :], start=True, stop=True
                        )
                        nc.tensor.matmul(
                            pet[32 * q : 32 * (q + 1), :],
                            kq[:, q, 128:160], qq[:, q, :],
                            start=True, stop=True, tile_position=(0, 32 * q),
                        )
                    p01 = st.tile([128, 2, H], BF16, name="p01", tag="p01")
                    p23 = st.tile([128, 2, H], BF16, name="p23", tag="p23")
                    pt = st.tile([128, H], BF16, name="pt", tag="pt")
                    nc.scalar.activation(p01[:], pe01[:], AF.Exp)
                    nc.scalar.activation(p23[:], pe23[:], AF.Exp)
                    nc.scalar.activation(pt[:], pet[:], AF.Exp)
                    if masked:
                        nc.vector.tensor_mul(p01[:, 0, :], p01[:, 0, :], mkm[:])
                        nc.vector.tensor_mul(p01[:, 1, :], p01[:, 1, :], mkm[:])
                        nc.vector.tensor_mul(p23[:, 0, :], p23[:, 0, :], mkm[:])
                        nc.vector.tensor_mul(p23[:, 1, :], p23[:, 1, :], mkm[:])
                        nc.vector.tensor_mul(pt[:], pt[:], mkt[:])

                    # PV: numerators + sums -> por [128, 5, CV]
                    por_b = ps.tile([128, 512], FP32, name="por_b", tag="por", bufs=2)
                    por = por_b[:, 0 : 5 * CV].rearrange("p (n c) -> p n c", c=CV)
                    for q in range(4):
                        pq = (p01, p23)[q // 2][:, q % 2, :]
                        sl = slice(32 * q, 32 * (q + 1))
                        for (w0, w1, cch, p0) in PIECES[q]:
                            dst = por[p0 : p0 + (w1 - w0), cch, :]
                            nc.tensor.matmul(
                                dst, pq[:, w0:w1], vm[:, q, :],
                                start=True, stop=False, tile_position=(0, p0),
                            )
                            nc.tensor.matmul(
                                dst, pt[sl, w0:w1], vt[sl, :],
                                start=False, stop=True, tile_position=(32 * q, p0),
                            )
                    finish(a, por)

            # ---- pass 1: column (H) attention on transposed image ----
            def finish_col(a, por):
                # quad rows are IMAGE COLUMNS w=4a..4a+3; positions along h.
                # stage to oc_d[h, w, :] (transposed scatter).
                oc_s = io.tile([128, 5, CV], FP32, name="oc_s", tag="oc_s")
                nc.vector.tensor_copy(oc_s[:], por[:])
                for q in range(4):
                    w = 4 * a + q
                    for (h0, h1, cch, p0) in PIECES[q]:
                        nc.sync.dma_start(
                            oc_d[h0:h1, w, :],
                            oc_s[p0 : p0 + (h1 - h0), cch, :],
                        )

            attention_pass(xTt_d, qTt_d, kTt_d, masked=True, finish=finish_col)

            # ---- pass 2: row (W) attention + combine ----
            def finish_row(a, por):
                base = 4 * a * 160
                ocq = io.tile([128, 5, CV], FP32, name="ocq", tag="ocq")
                xf = io.tile([128, 5, C], FP32, name="xf", tag="xf")
                for cch in range(5):
                    off = (base + cch * 128) * CV
                    nc.sync.dma_start(
                        ocq[:, cch, :],
                        oc_flat[off : off + 128 * CV].rearrange("(p j) -> p j", j=CV),
                    )
                    nc.sync.dma_start(
                        xf[:, cch, :],
                        x_d[base + cch * 128 : base + (cch + 1) * 128, :],
                    )
                num = io.tile([128, 5, CV], FP32, name="num", tag="num")
                nc.vector.tensor_add(num[:], por[:], ocq[:])
                rec = io.tile([128, 5], FP32, name="rec", tag="rec")
                nc.vector.reciprocal(rec[:], num[:, :, C])
                res = io.tile([128, 5, C], FP32, name="res", tag="res")
                for cch in range(5):
                    nc.vector.tensor_scalar_mul(
                        res[:, cch, :], num[:, cch, 0:C], rec[:, cch : cch + 1]
                    )
                nc.vector.tensor_add(res[:], res[:], xf[:])
                for cch in range(5):
                    nc.sync.dma_start(
                        out_d[base + cch * 128 : base + (cch + 1) * 128, :],
                        res[:, cch, :],
                    )

            attention_pass(xT_d, qT_d, kT_d, masked=False, finish=finish_row)

    nc.compile()
    _cache["nc"] = nc
    return nc


def _host_prep(x, Wq, Wk, Wv, gamma):
    bf16 = mybir.dt.np(BF16)
    wvg = (Wv * float(np.asarray(gamma).reshape(-1)[0])).astype(bf16)
    eye_c = 1.0 - np.eye(H, dtype=np.float32)
    mask_main = eye_c[0:128, :].astype(bf16)
    mask_tail = np.concatenate([eye_c[128:H, :]] * 4, axis=0).astype(bf16)
    in_maps = []
    for b in range(B):
        xb = np.asarray(x[b], dtype=np.float32).reshape(NPOS, C)  # (h w) c
        xbt = np.asarray(x[b], dtype=np.float32).transpose(1, 0, 2).reshape(NPOS, C)
        q = xb @ Wq
        k = xb @ Wk
        qt = xbt @ Wq
        kt = xbt @ Wk
        in_maps.append(
            {
                "x_in": xb,
                "xT_in": np.ascontiguousarray(xb.T).astype(bf16),
                "xTt_in": np.ascontiguousarray(xbt.T).astype(bf16),
                "qT_in": np.ascontiguousarray(q.T).astype(bf16),
                "kT_in": np.ascontiguousarray(k.T).astype(bf16),
                "qTt_in": np.ascontiguousarray(qt.T).astype(bf16),
                "kTt_in": np.ascontiguousarray(kt.T).astype(bf16),
                "wv": wvg,
                "mask_main": mask_main,
                "mask_tail": mask_tail,
            }
        )
    return in_maps


def kernel(x, Wq, Wk, Wv, gamma, **kw):
    nc = _build()
    in_maps = _host_prep(
        np.asarray(x, np.float32),
        np.asarray(Wq, np.float32),
        np.asarray(Wk, np.float32),
        np.asarray(Wv, np.float32),
        np.asarray(gamma, np.float32),
    )
    res = bass_utils.run_bass_kernel_spmd(nc, in_maps, core_ids=list(range(B)))
    out = np.stack([res.results[b]["out"].reshape(H, W, C) for b in range(B)])
    return out.astype(np.float32)


if __name__ == "__main__":
    rng = np.random.default_rng(0)
    x = rng.standard_normal((B, H, W, C), dtype=np.float32)
    Wq = rng.standard_normal((C, CQ), dtype=np.float32) * (2.0 / C) ** 0.5
    Wk = rng.standard_normal((C, CQ), dtype=np.float32) * (2.0 / C) ** 0.5
    Wv = rng.standard_normal((C, C), dtype=np.float32) * (2.0 / C) ** 0.5
    gamma = np.array([0.1], dtype=np.float32)
    o = kernel(x=x, Wq=Wq, Wk=Wk, Wv=Wv, gamma=gamma)
    print(o.shape, o.dtype)


# revision 13
# speedup vs baseline: 11141.7502x; 11141.7502x over previous
"""Criss-cross attention TRN2 kernel.

B=8 images, one per NeuronCore (data-parallel over batch).
H=W=160, C=64, CQ=8. Joint softmax over row+column scores computed as two
identical "row attention" passes — the column pass runs on the spatially
transposed image (with the diagonal mask) and stages its numerators/sums
to DRAM; the row pass adds them, normalizes, adds x. Softmax without
max-subtraction (|E| < ~40 is safe in fp32/bf16). gamma folded into Wv.

All SBUF working tiles are streamed per quad (4 rows), so SBUF stays tiny.
Scores/PV matmuls in bf16; accumulation fp32 in PSUM.
"""

import sys

for p in ("/opt/trn_rl_repo", "/root/.axon_site/_ro/trn_rl_repo"):
    if p not in sys.path:
        sys.path.insert(0, p)

import numpy as np

import concourse.bacc as bacc
import concourse.bass as bass
import concourse.tile as tile
from concourse import mybir
from concourse import bass_utils

B, H, W, C = 8, 160, 160, 64
CQ = 8
NPOS = H * W
CV = C + 1  # v channels + ones column (sum-of-P trick)
NQ = H // 4  # 40 quads of 4 rows

FP32 = mybir.dt.float32
BF16 = mybir.dt.bfloat16
AF = mybir.ActivationFunctionType

# quad-relative output pieces: row q of a quad covers quad-positions
# q*160..q*160+159 -> (w0, w1, chunk, part0) with part0 in {0,32,64,96}
PIECES = {
    0: [(0, 128, 0, 0), (128, 160, 1, 0)],
    1: [(0, 32, 1, 32), (32, 96, 1, 64), (96, 160, 2, 0)],
    2: [(0, 64, 2, 64), (64, 160, 3, 0)],
    3: [(0, 32, 3, 96), (32, 160, 4, 0)],
}

_cache = {}


def _build():
    if "nc" in _cache:
        return _cache["nc"]

    nc = bacc.Bacc("TRN2", target_bir_lowering=False, debug=False)

    # pass-2 (row) inputs: (h w)-major; pass-1 (col) inputs: (w h)-major
    x_d = nc.dram_tensor("x_in", [NPOS, C], FP32, kind="ExternalInput").ap()
    xT_d = nc.dram_tensor("xT_in", [C, NPOS], BF16, kind="ExternalInput").ap()
    xTt_d = nc.dram_tensor("xTt_in", [C, NPOS], BF16, kind="ExternalInput").ap()
    qT_d = nc.dram_tensor("qT_in", [CQ, NPOS], BF16, kind="ExternalInput").ap()
    kT_d = nc.dram_tensor("kT_in", [CQ, NPOS], BF16, kind="ExternalInput").ap()
    qTt_d = nc.dram_tensor("qTt_in", [CQ, NPOS], BF16, kind="ExternalInput").ap()
    kTt_d = nc.dram_tensor("kTt_in", [CQ, NPOS], BF16, kind="ExternalInput").ap()
    wv_d = nc.dram_tensor("wv", [C, C], BF16, kind="ExternalInput").ap()
    mka_d = nc.dram_tensor("mask_a", [128, 3 * H], BF16, kind="ExternalInput").ap()
    mkb_d = nc.dram_tensor("mask_b", [128, 3 * H], BF16, kind="ExternalInput").ap()
    out_d = nc.dram_tensor("out", [NPOS, C], FP32, kind="ExternalOutput").ap()
    # column-pass staging, (h w)-major so the row pass reads it contiguously
    oc_d = nc.dram_tensor("oc_stage", [H, W, CV], BF16, kind="Internal").ap()
    oc_flat = oc_d.rearrange("h w c -> (h w c)")

    with tile.TileContext(nc) as tc:
        with (
            tc.tile_pool(name="cst", bufs=1) as cst,
            tc.tile_pool(name="st", bufs=3) as st,
            tc.tile_pool(name="io", bufs=2) as io,
            tc.tile_pool(name="ps", bufs=1, space="PSUM") as ps,
        ):
            wv = cst.tile([C, C], BF16)
            nc.sync.dma_start(wv[:], wv_d[:])
            mka = cst.tile([128, 3 * H], BF16)
            nc.sync.dma_start(mka[:], mka_d[:])
            mkb = cst.tile([128, 3 * H], BF16)
            nc.sync.dma_start(mkb[:], mkb_d[:])

            def attention_pass(xT_src, qT_src, kT_src, masked, finish):
                """One direction of criss-cross attention.

                Per quad a (4 "rows" of 160 positions): build V tiles for
                these rows, scores E^T = K^T-chunks x Q, exp (bf16), PV with
                ones-column giving numerators + sums in PSUM [128,5,CV];
                finish(a, por) consumes the PSUM quad.
                """
                for a in range(NQ):
                    base = 4 * a * 160
                    if a % 4 == 0:
                        # stream 4 quads (16 rows) of inputs at once
                        gb = base
                        xg_f = st.tile([C, 2560], BF16, name="xg_f", tag="xq")
                        nc.sync.dma_start(xg_f[:], xT_src[:, gb : gb + 2560])
                        xg = xg_f.rearrange("c (n p) -> c n p", p=160)
                        qg_f = st.tile([CQ, 2560], BF16, name="qg_f", tag="qq")
                        nc.sync.dma_start(qg_f[:], qT_src[:, gb : gb + 2560])
                        qg = qg_f.rearrange("c (n p) -> c n p", p=160)
                        kg_f = st.tile([CQ, 2560], BF16, name="kg_f", tag="kq")
                        nc.sync.dma_start(kg_f[:], kT_src[:, gb : gb + 2560])
                        kg = kg_f.rearrange("c (n p) -> c n p", p=160)
                    al = 4 * (a % 4)
                    xq = xg[:, al : al + 4, :]
                    qq = qg[:, al : al + 4, :]
                    kq = kg[:, al : al + 4, :]

                    # V for the 4 rows: main [128,4,CV] + packed tails [128,CV]
                    pv_b = ps.tile([128, 512], FP32, name="pv_b", tag="pv", bufs=2)
                    pv = pv_b[:, 0 : 4 * C + C]
                    for q in range(4):
                        nc.tensor.matmul(
                            pv[:, q * C : (q + 1) * C], xq[:, q, 0:128], wv[:],
                            start=True, stop=True,
                        )
                        nc.tensor.matmul(
                            pv[32 * q : 32 * (q + 1), 4 * C : 5 * C],
                            xq[:, q, 128:160], wv[:],
                            start=True, stop=True, tile_position=(0, 32 * q),
                        )
                    vm = st.tile([128, 4, CV], BF16, name="vm", tag="vm")
                    vt = st.tile([128, CV], BF16, name="vt", tag="vt")
                    pv5 = pv[:, 0 : 4 * C].rearrange("p (q c) -> p q c", c=C)
                    nc.vector.tensor_copy(vm[:, :, 0:C], pv5[:])
                    nc.vector.tensor_copy(vt[:, 0:C], pv[:, 4 * C : 5 * C])
                    nc.vector.memset(vm[:, :, C], 1.0)
                    nc.vector.memset(vt[:, C : C + 1], 1.0)

                    # scores: E^T [k, pos]; rows 2-per-bank, tails packed
                    # into cols 320:480 at partitions 32q
                    pe01_b = ps.tile([128, 512], FP32, name="pe01_b", tag="pe01", bufs=2)
                    pe23_b = ps.tile([128, 512], FP32, name="pe23_b", tag="pe23", bufs=2)
                    for q in range(4):
                        pb = (pe01_b, pe23_b)[q // 2]
                        nc.tensor.matmul(
                            pb[:, (q % 2) * H : (q % 2 + 1) * H],
                            kq[:, q, 0:128], qq[:, q, :], start=True, stop=True,
                        )
                        # all four tails packed into pe01 cols 320:480
                        nc.tensor.matmul(
                            pe01_b[32 * q : 32 * (q + 1), 2 * H : 3 * H],
                            kq[:, q, 128:160], qq[:, q, :],
                            start=True, stop=True, tile_position=(0, 32 * q),
                        )
                    p01 = st.tile([128, 3 * H], BF16, name="p01", tag="p01")
                    p23 = st.tile([128, 2 * H], BF16, name="p23", tag="p23")
                    nc.scalar.activation(p01[:], pe01_b[:, 0 : 3 * H], AF.Exp)
                    nc.scalar.activation(p23[:], pe23_b[:, 0 : 2 * H], AF.Exp)
                    if masked:
                        nc.vector.tensor_mul(p01[:], p01[:], mka[:])
                        nc.vector.tensor_mul(p23[:], p23[:], mka[:, 0 : 2 * H])

                    # PV: numerators + sums -> por [128, 5, CV]
                    por_b = ps.tile([128, 512], FP32, name="por_b", tag="por", bufs=2)
                    por = por_b[:, 0 : 5 * CV].rearrange("p (n c) -> p n c", c=CV)
                    for q in range(4):
                        pfull = (p01, p23)[q // 2]
                        pq = pfull[:, (q % 2) * H : (q % 2 + 1) * H]
                        sl = slice(32 * q, 32 * (q + 1))
                        for (w0, w1, cch, p0) in PIECES[q]:
                            dst = por[p0 : p0 + (w1 - w0), cch, :]
                            nc.tensor.matmul(
                                dst, pq[:, w0:w1], vm[:, q, :],
                                start=True, stop=False, tile_position=(0, p0),
                            )
                            nc.tensor.matmul(
                                dst, p01[sl, 2 * H + w0 : 2 * H + w1], vt[sl, :],
                                start=False, stop=True, tile_position=(32 * q, p0),
                            )
                    finish(a, por)

            # ---- pass 1: column (H) attention on transposed image ----
            def finish_col(a, por):
                # quad rows are IMAGE COLUMNS w=4a..4a+3; positions along h.
                # stage to oc_d[h, w, :] (transposed scatter).
                oc_s = io.tile([128, 5, CV], BF16, name="oc_s", tag="oc_s")
                nc.vector.tensor_copy(oc_s[:], por[:])
                for q in range(4):
                    w = 4 * a + q
                    eng = nc.sync if q < 2 else nc.gpsimd
                    for (h0, h1, cch, p0) in PIECES[q]:
                        eng.dma_start(
                            oc_d[h0:h1, w, :],
                            oc_s[p0 : p0 + (h1 - h0), cch, :],
                        )

            attention_pass(xTt_d, qTt_d, kTt_d, masked=True, finish=finish_col)

            # ---- pass 2: row (W) attention + combine ----
            def finish_row(a, por):
                base = 4 * a * 160
                ocq = io.tile([128, 5, CV], BF16, name="ocq", tag="ocq")
                xf = io.tile([128, 5, C], FP32, name="xf", tag="xf")
                # single DMAs: DRAM side reordered so partition dim is outer
                nc.sync.dma_start(
                    ocq[:],
                    oc_flat[base * CV : (base + 640) * CV].rearrange(
                        "(c p j) -> p c j", p=128, j=CV
                    ),
                )
                nc.sync.dma_start(
                    xf[:],
                    x_d[base : base + 640, :].rearrange("(c p) j -> p c j", p=128),
                )
                num = io.tile([128, 5, CV], FP32, name="num", tag="num")
                nc.vector.tensor_add(num[:], por[:], ocq[:])
                rec = io.tile([128, 5], FP32, name="rec", tag="rec")
                nc.vector.reciprocal(rec[:], num[:, :, C])
                res = io.tile([128, 5, C], FP32, name="res", tag="res")
                for cch in range(5):
                    nc.vector.tensor_scalar_mul(
                        res[:, cch, :], num[:, cch, 0:C], rec[:, cch : cch + 1]
                    )
                nc.vector.tensor_add(res[:], res[:], xf[:])
                nc.sync.dma_start(
                    out_d[base : base + 640, :].rearrange("(c p) j -> p c j", p=128),
                    res[:],
                )

            attention_pass(xT_d, qT_d, kT_d, masked=False, finish=finish_row)

    nc.compile()
    _cache["nc"] = nc
    return nc


def _host_prep(x, Wq, Wk, Wv, gamma):
    bf16 = mybir.dt.np(BF16)
    wvg = (Wv * float(np.asarray(gamma).reshape(-1)[0])).astype(bf16)
    eye_c = 1.0 - np.eye(H, dtype=np.float32)
    mkm = eye_c[0:128, :]
    ta = np.ones((128, H), dtype=np.float32)
    ta[0:32] = eye_c[128:H, :]
    ta[32:64] = eye_c[128:H, :]
    tb = np.ones((128, H), dtype=np.float32)
    tb[64:96] = eye_c[128:H, :]
    tb[96:128] = eye_c[128:H, :]
    mask_a = np.concatenate([mkm, mkm, ta], axis=1).astype(bf16)
    mask_b = np.concatenate([mkm, mkm, tb], axis=1).astype(bf16)
    in_maps = []
    for b in range(B):
        xb = np.asarray(x[b], dtype=np.float32).reshape(NPOS, C)  # (h w) c
        xbt = np.asarray(x[b], dtype=np.float32).transpose(1, 0, 2).reshape(NPOS, C)
        q = xb @ Wq
        k = xb @ Wk
        qt = xbt @ Wq
        kt = xbt @ Wk
        in_maps.append(
            {
                "x_in": xb,
                "xT_in": np.ascontiguousarray(xb.T).astype(bf16),
                "xTt_in": np.ascontiguousarray(xbt.T).astype(bf16),
                "qT_in": np.ascontiguousarray(q.T).astype(bf16),
                "kT_in": np.ascontiguousarray(k.T).astype(bf16),
                "qTt_in": np.ascontiguousarray(qt.T).astype(bf16),
                "kTt_in": np.ascontiguousarray(kt.T).astype(bf16),
                "wv": wvg,
                "mask_a": mask_a,
                "mask_b": mask_b,
            }
        )
    return in_maps


def kernel(x, Wq, Wk, Wv, gamma, **kw):
    nc = _build()
    in_maps = _host_prep(
        np.asarray(x, np.float32),
        np.asarray(Wq, np.float32),
        np.asarray(Wk, np.float32),
        np.asarray(Wv, np.float32),
        np.asarray(gamma, np.float32),
    )
    res = bass_utils.run_bass_kernel_spmd(nc, in_maps, core_ids=list(range(B)))
    out = np.stack([res.results[b]["out"].reshape(H, W, C) for b in range(B)])
    return out.astype(np.float32)


if __name__ == "__main__":
    rng = np.random.default_rng(0)
    x = rng.standard_normal((B, H, W, C), dtype=np.float32)
    Wq = rng.standard_normal((C, CQ), dtype=np.float32) * (2.0 / C) ** 0.5
    Wk = rng.standard_normal((C, CQ), dtype=np.float32) * (2.0 / C) ** 0.5
    Wv = rng.standard_normal((C, C), dtype=np.float32) * (2.0 / C) ** 0.5
    gamma = np.array([0.1], dtype=np.float32)
    o = kernel(x=x, Wq=Wq, Wk=Wk, Wv=Wv, gamma=gamma)
    print(o.shape, o.dtype)


# revision 16
# speedup vs baseline: 18453.3760x; 1.6562x over previous
"""Criss-cross attention TRN2 kernel.

B=8 images, one per NeuronCore (data-parallel over batch).
H=W=160, C=64, CQ=8. Joint softmax over row+column scores computed as two
identical "row attention" passes — the column pass runs on the spatially
transposed image (with the diagonal mask) and stages its numerators/sums
to DRAM; the row pass adds them, normalizes, adds x. Softmax without
max-subtraction (|E| < ~40 is safe in fp32/bf16). gamma folded into Wv.

All SBUF working tiles are streamed per quad (4 rows), so SBUF stays tiny.
Scores/PV matmuls in bf16; accumulation fp32 in PSUM.
"""

import sys

for p in ("/opt/trn_rl_repo", "/root/.axon_site/_ro/trn_rl_repo"):
    if p not in sys.path:
        sys.path.insert(0, p)

import numpy as np

import concourse.bacc as bacc
import concourse.bass as bass
import concourse.tile as tile
from concourse import mybir
from concourse import bass_utils

B, H, W, C = 8, 160, 160, 64
CQ = 8
NPOS = H * W
CV = C + 1  # v channels + ones column (sum-of-P trick)
NQ = H // 4  # 40 quads of 4 rows

FP32 = mybir.dt.float32
BF16 = mybir.dt.bfloat16
AF = mybir.ActivationFunctionType

# quad-relative output pieces: row q of a quad covers quad-positions
# q*160..q*160+159 -> (w0, w1, chunk, part0) with part0 in {0,32,64,96}
PIECES = {
    0: [(0, 128, 0, 0), (128, 160, 1, 0)],
    1: [(0, 32, 1, 32), (32, 96, 1, 64), (96, 160, 2, 0)],
    2: [(0, 64, 2, 64), (64, 160, 3, 0)],
    3: [(0, 32, 3, 96), (32, 160, 4, 0)],
}

_cache = {}


def _build():
    if "nc" in _cache:
        return _cache["nc"]

    nc = bacc.Bacc("TRN2", target_bir_lowering=False, debug=False)

    # pass-2 (row) inputs: (h w)-major; pass-1 (col) inputs: (w h)-major
    x_d = nc.dram_tensor("x_in", [NPOS, C], FP32, kind="ExternalInput").ap()
    xT_d = nc.dram_tensor("xT_in", [C, NPOS], BF16, kind="ExternalInput").ap()
    xTt_d = nc.dram_tensor("xTt_in", [C, NPOS], BF16, kind="ExternalInput").ap()
    qT_d = nc.dram_tensor("qT_in", [CQ, NPOS], BF16, kind="ExternalInput").ap()
    kT_d = nc.dram_tensor("kT_in", [CQ, NPOS], BF16, kind="ExternalInput").ap()
    qTt_d = nc.dram_tensor("qTt_in", [CQ, NPOS], BF16, kind="ExternalInput").ap()
    kTt_d = nc.dram_tensor("kTt_in", [CQ, NPOS], BF16, kind="ExternalInput").ap()
    wv_d = nc.dram_tensor("wv", [C, C], BF16, kind="ExternalInput").ap()
    mka_d = nc.dram_tensor("mask_a", [128, 3 * H], BF16, kind="ExternalInput").ap()
    mkb_d = nc.dram_tensor("mask_b", [128, 3 * H], BF16, kind="ExternalInput").ap()
    out_d = nc.dram_tensor("out", [NPOS, C], FP32, kind="ExternalOutput").ap()
    # column-pass staging, (h w)-major so the row pass reads it contiguously
    oc_d = nc.dram_tensor("oc_stage", [H, W, CV], BF16, kind="Internal").ap()
    oc_flat = oc_d.rearrange("h w c -> (h w c)")

    with tile.TileContext(nc) as tc:
        with (
            tc.tile_pool(name="cst", bufs=1) as cst,
            tc.tile_pool(name="st", bufs=3) as st,
            tc.tile_pool(name="io", bufs=3) as io,
            tc.tile_pool(name="ps", bufs=1, space="PSUM") as ps,
        ):
            wv = cst.tile([C, C], BF16)
            nc.sync.dma_start(wv[:], wv_d[:])
            mka = cst.tile([128, 3 * H], BF16)
            nc.sync.dma_start(mka[:], mka_d[:])
            mkb = cst.tile([128, 3 * H], BF16)
            nc.sync.dma_start(mkb[:], mkb_d[:])

            def attention_pass(xT_src, qT_src, kT_src, masked, finish):
                """One direction of criss-cross attention.

                Per quad a (4 "rows" of 160 positions): build V tiles for
                these rows, scores E^T = K^T-chunks x Q, exp (bf16), PV with
                ones-column giving numerators + sums in PSUM [128,5,CV];
                finish(a, por) consumes the PSUM quad.
                """
                # q/k whole-pass resident; x streamed per 4 quads
                qg_f = st.tile([CQ, NPOS], BF16, name="qg_f", tag="qq", bufs=1)
                nc.sync.dma_start(qg_f[:], qT_src[:])
                qg = qg_f.rearrange("c (n p) -> c n p", p=160)
                kg_f = st.tile([CQ, NPOS], BF16, name="kg_f", tag="kq", bufs=1)
                nc.sync.dma_start(kg_f[:], kT_src[:])
                kg = kg_f.rearrange("c (n p) -> c n p", p=160)
                for a in range(NQ):
                    base = 4 * a * 160
                    if a % 4 == 0:
                        xg_f = st.tile([C, 2560], BF16, name="xg_f", tag="xq")
                        nc.sync.dma_start(xg_f[:], xT_src[:, base : base + 2560])
                        xg = xg_f.rearrange("c (n p) -> c n p", p=160)
                    al = 4 * (a % 4)
                    xq = xg[:, al : al + 4, :]
                    qq = qg[:, 4 * a : 4 * a + 4, :]
                    kq = kg[:, 4 * a : 4 * a + 4, :]

                    # V for the 4 rows: main [128,4,CV] + packed tails [128,CV]
                    pv_b = ps.tile([128, 512], FP32, name="pv_b", tag="pv", bufs=2)
                    pv = pv_b[:, 0 : 4 * C + C]
                    for q in range(4):
                        nc.tensor.matmul(
                            pv[:, q * C : (q + 1) * C], xq[:, q, 0:128], wv[:],
                            start=True, stop=True,
                        )
                        nc.tensor.matmul(
                            pv[32 * q : 32 * (q + 1), 4 * C : 5 * C],
                            xq[:, q, 128:160], wv[:],
                            start=True, stop=True, tile_position=(0, 32 * q),
                        )
                    vm = st.tile([128, 4, CV], BF16, name="vm", tag="vm")
                    vt = st.tile([128, CV], BF16, name="vt", tag="vt")
                    pv5 = pv[:, 0 : 4 * C].rearrange("p (q c) -> p q c", c=C)
                    nc.scalar.activation(vm[:, :, 0:C], pv5[:], AF.Copy)
                    nc.scalar.activation(vt[:, 0:C], pv[:, 4 * C : 5 * C], AF.Copy)
                    nc.vector.memset(vm[:, :, C], 1.0)
                    nc.vector.memset(vt[:, C : C + 1], 1.0)

                    # scores: E^T [k, pos]; rows 2-per-bank, tails packed
                    # into cols 320:480 at partitions 32q
                    pe01_b = ps.tile([128, 512], FP32, name="pe01_b", tag="pe01", bufs=2)
                    pe23_b = ps.tile([128, 512], FP32, name="pe23_b", tag="pe23", bufs=2)
                    for q in range(4):
                        pb = (pe01_b, pe23_b)[q // 2]
                        nc.tensor.matmul(
                            pb[:, (q % 2) * H : (q % 2 + 1) * H],
                            kq[:, q, 0:128], qq[:, q, :], start=True, stop=True,
                        )
                        # all four tails packed into pe01 cols 320:480
                        nc.tensor.matmul(
                            pe01_b[32 * q : 32 * (q + 1), 2 * H : 3 * H],
                            kq[:, q, 128:160], qq[:, q, :],
                            start=True, stop=True, tile_position=(0, 32 * q),
                        )
                    p01 = st.tile([128, 3 * H], BF16, name="p01", tag="p01")
                    p23 = st.tile([128, 2 * H], BF16, name="p23", tag="p23")
                    nc.scalar.activation(p01[:], pe01_b[:, 0 : 3 * H], AF.Exp)
                    nc.scalar.activation(p23[:], pe23_b[:, 0 : 2 * H], AF.Exp)
                    if masked:
                        nc.vector.tensor_mul(p01[:], p01[:], mka[:])
                        nc.vector.tensor_mul(p23[:], p23[:], mka[:, 0 : 2 * H])

                    # PV: numerators + sums -> por [128, 5, CV]
                    por_b = ps.tile([128, 512], FP32, name="por_b", tag="por", bufs=2)
                    por = por_b[:, 0 : 5 * CV].rearrange("p (n c) -> p n c", c=CV)
                    for q in range(4):
                        pfull = (p01, p23)[q // 2]
                        pq = pfull[:, (q % 2) * H : (q % 2 + 1) * H]
                        sl = slice(32 * q, 32 * (q + 1))
                        for (w0, w1, cch, p0) in PIECES[q]:
                            dst = por[p0 : p0 + (w1 - w0), cch, :]
                            nc.tensor.matmul(
                                dst, pq[:, w0:w1], vm[:, q, :],
                                start=True, stop=False, tile_position=(0, p0),
                            )
                            nc.tensor.matmul(
                                dst, p01[sl, 2 * H + w0 : 2 * H + w1], vt[sl, :],
                                start=False, stop=True, tile_position=(32 * q, p0),
                            )
                    finish(a, por)

            # ---- pass 1: column (H) attention on transposed image ----
            # whole-image staging tiles (bf16, 26KB each)
            ocall = cst.tile([128, NQ, 5, CV], BF16)
            ocall2 = cst.tile([128, 5 * NQ, CV], BF16)

            def finish_col(a, por):
                # quad rows are IMAGE COLUMNS w=4a..4a+3; positions along h
                nc.vector.tensor_copy(ocall[:, a, :, :], por[:])

            attention_pass(xTt_d, qTt_d, kTt_d, masked=True, finish=finish_col)

            # transpose through DRAM: 9 piece-DMAs spanning all 40 col-quads,
            # then one whole-image readback in row-major chunk order
            oc_d3 = oc_d.rearrange("h (a q) c -> h a q c", q=4)
            for q in range(4):
                for (h0, h1, cch, p0) in PIECES[q]:
                    nc.sync.dma_start(
                        oc_d3[h0:h1, :, q, :],
                        ocall[p0 : p0 + (h1 - h0), :, cch, :],
                    )
            nc.sync.dma_start(
                ocall2[:],
                oc_flat[:].rearrange("(c p j) -> p c j", p=128, j=CV),
            )

            # ---- pass 2: row (W) attention + combine ----
            def finish_row(a, por):
                base = 4 * a * 160
                ocq = ocall2[:, 5 * a : 5 * a + 5, :]
                xf = io.tile([128, 5, C], FP32, name="xf", tag="xf")
                nc.sync.dma_start(
                    xf[:],
                    x_d[base : base + 640, :].rearrange("(c p) j -> p c j", p=128),
                )
                num = io.tile([128, 5, CV], FP32, name="num", tag="num")
                nc.vector.tensor_add(num[:], por[:], ocq[:])
                rec = io.tile([128, 5], FP32, name="rec", tag="rec")
                nc.vector.reciprocal(rec[:], num[:, :, C])
                res = io.tile([128, 5, C], FP32, name="res", tag="res")
                for cch in range(5):
                    nc.vector.tensor_scalar_mul(
                        res[:, cch, :], num[:, cch, 0:C], rec[:, cch : cch + 1]
                    )
                nc.gpsimd.tensor_add(res[:], res[:], xf[:])
                nc.sync.dma_start(
                    out_d[base : base + 640, :].rearrange("(c p) j -> p c j", p=128),
                    res[:],
                )

            attention_pass(xT_d, qT_d, kT_d, masked=False, finish=finish_row)

    nc.compile()
    _cache["nc"] = nc
    return nc


def _host_prep(x, Wq, Wk, Wv, gamma):
    bf16 = mybir.dt.np(BF16)
    wvg = (Wv * float(np.asarray(gamma).reshape(-1)[0])).astype(bf16)
    eye_c = 1.0 - np.eye(H, dtype=np.float32)
    mkm = eye_c[0:128, :]
    ta = np.ones((128, H), dtype=np.float32)
    ta[0:32] = eye_c[128:H, :]
    ta[32:64] = eye_c[128:H, :]
    tb = np.ones((128, H), dtype=np.float32)
    tb[64:96] = eye_c[128:H, :]
    tb[96:128] = eye_c[128:H, :]
    mask_a = np.concatenate([mkm, mkm, ta], axis=1).astype(bf16)
    mask_b = np.concatenate([mkm, mkm, tb], axis=1).astype(bf16)
    in_maps = []
    for b in range(B):
        xb = np.asarray(x[b], dtype=np.float32).reshape(NPOS, C)  # (h w) c
        xbt = np.asarray(x[b], dtype=np.float32).transpose(1, 0, 2).reshape(NPOS, C)
        q = xb @ Wq
        k = xb @ Wk
        qt = xbt @ Wq
        kt = xbt @ Wk
        in_maps.append(
            {
                "x_in": xb,
                "xT_in": np.ascontiguousarray(xb.T).astype(bf16),
                "xTt_in": np.ascontiguousarray(xbt.T).astype(bf16),
                "qT_in": np.ascontiguousarray(q.T).astype(bf16),
                "kT_in": np.ascontiguousarray(k.T).astype(bf16),
                "qTt_in": np.ascontiguousarray(qt.T).astype(bf16),
                "kTt_in": np.ascontiguousarray(kt.T).astype(bf16),
                "wv": wvg,
                "mask_a": mask_a,
                "mask_b": mask_b,
            }
        )
    return in_maps


def kernel(x, Wq, Wk, Wv, gamma, **kw):
    nc = _build()
    in_maps = _host_prep(
        np.asarray(x, np.float32),
        np.asarray(Wq, np.float32),
        np.asarray(Wk, np.float32),
        np.asarray(Wv, np.float32),
        np.asarray(gamma, np.float32),
    )
    res = bass_utils.run_bass_kernel_spmd(nc, in_maps, core_ids=list(range(B)))
    out = np.stack([res.results[b]["out"].reshape(H, W, C) for b in range(B)])
    return out.astype(np.float32)


if __name__ == "__main__":
    rng = np.random.default_rng(0)
    x = rng.standard_normal((B, H, W, C), dtype=np.float32)
    Wq = rng.standard_normal((C, CQ), dtype=np.float32) * (2.0 / C) ** 0.5
    Wk = rng.standard_normal((C, CQ), dtype=np.float32) * (2.0 / C) ** 0.5
    Wv = rng.standard_normal((C, C), dtype=np.float32) * (2.0 / C) ** 0.5
    gamma = np.array([0.1], dtype=np.float32)
    o = kernel(x=x, Wq=Wq, Wk=Wk, Wv=Wv, gamma=gamma)
    print(o.shape, o.dtype)
